# revision 21
# baseline (speedup 1.0000x reference)
"""FP8Linear forward on 8 Trainium2 NeuronCores (Bass/Tile).

Computes out[b,s,o] = sum_i bf16(x)[b,s,i] * q[o,i] * scale[o] + bias[o]
where q = weight_fp8 (fp8-representable values stored as f32).

Sharding: tensor-parallel over out_features (column parallel). Each of the
8 cores owns a 512-row slice of the weight and computes the full token range
for its output slice; x is replicated.

Device kernel layout (per core):
  - lhsT (stationary) = qT tile [i=128, o=128]   (weight slice, pre-transposed)
  - rhs  (moving)     = xT tile [i=128, t=512]   (x, pre-transposed on host)
  - PSUM accumulates out^T tile [o=128, t=512] f32 over 32 i-chunks
  - ScalarE epilogue: out = Identity(psum * scale[o] + bias[o]) -> bf16
    (dequant scale folded into the epilogue; scale is per-out-channel so it
     lands on PSUM partitions, broadcast along the free/t dim)

Host does layout marshaling only: f32->bf16 casts (bf16(x) is the reference's
own first step; weight_fp8 values are exactly representable in bf16) and
transposes so every device DMA is contiguous-per-partition.
"""

import numpy as np
import ml_dtypes
from contextlib import ExitStack

import concourse.bass as bass
from concourse import bacc
import concourse.mybir as mybir
import concourse.tile as tile
from concourse.bass import ts
from concourse.bass_utils import run_bass_kernel_spmd

BF16 = ml_dtypes.bfloat16

P = 128
B, S, IN, OUT = 2, 2048, 4096, 4096
T = B * S                 # 4096 tokens
NCORES = 8
O_C = OUT // NCORES       # 512 out-features per core
KI = IN // P              # 32 contraction chunks
T_TILE = 512
T_TILES = T // T_TILE     # 8
O_TILES = O_C // P        # 4
DMA_KC = 4                # i-chunks per DMA (512 KB x-chunks)

# Set by test.py to capture a profile; harness path leaves these alone.
TRACE = False
LAST_RESULT = None


def _build_bass():
    nc = bacc.Bacc("TRN2", target_bir_lowering=False, debug=False,
                   num_devices=NCORES)

    xT = nc.dram_tensor("xT", [IN, T], mybir.dt.bfloat16, kind="ExternalInput")
    qT = nc.dram_tensor("qT", [IN, O_C], mybir.dt.bfloat16, kind="ExternalInput")
    # scale and bias packed as [O_C, 2] so one DMA covers both (fewer sem
    # waits on the first Activation, which has a small HW wait budget).
    scbias = nc.dram_tensor("scbias", [O_C, 2], mybir.dt.float32, kind="ExternalInput")
    outT = nc.dram_tensor("outT", [O_C, T], mybir.dt.bfloat16, kind="ExternalOutput")

    # [i, f] -> [ki=128 partitions, ko, f]; per-partition rows are contiguous
    xT_t = xT.ap().rearrange("(ko ki) t -> ki ko t", ki=P)
    qT_t = qT.ap().rearrange("(ko ki) o -> ki ko o", ki=P)
    # [o, 2] -> [p=128 partitions, (o_tile, 2)]; per-o-tile scalars are [128, 1]
    sb_t = scbias.ap().rearrange("(ot p) two -> p ot two", p=P)

    with ExitStack() as ctx:
        tc = ctx.enter_context(tile.TileContext(nc))
        wpool = ctx.enter_context(tc.tile_pool(name="w", bufs=1))
        spool = ctx.enter_context(tc.tile_pool(name="s", bufs=1))
        xpool = ctx.enter_context(tc.tile_pool(name="x", bufs=2))
        # One slot per output tile: slot reuse would add a WAR wait on the
        # epilogue Activation, which can only encode a single sem wait.
        opool = ctx.enter_context(tc.tile_pool(name="o", bufs=T_TILES * O_TILES))
        pspool = ctx.enter_context(tc.tile_pool(name="ps", bufs=8, space="PSUM"))

        # Weight slice resident in SBUF for the whole kernel (4 MB bf16).
        # Interleave weight chunks with the first x tile's chunks (emitted in
        # the tt==0 iteration below) so PE can start as soon as matching
        # ki-chunks of both are resident.
        qt = wpool.tile([P, KI, O_C], mybir.dt.bfloat16)

        # The AC (Activation) instruction encodes only ONE sem wait, but the
        # epilogue activation depends on both the PSUM matmul group and the
        # scbias load. Stage scbias through two early ScalarE copies: copy1
        # carries the DMA wait; copy2 (a dummy read of sb) carries the
        # same-engine pipeline-hazard wait on copy1. After that, ScalarE's
        # observed clock covers sb, so every epilogue only waits on PE.
        sb_raw = spool.tile([P, O_TILES, 2], mybir.dt.float32)
        nc.gpsimd.dma_start(sb_raw[:], sb_t)
        sb = spool.tile([P, O_TILES, 2], mybir.dt.float32)
        nc.scalar.copy(sb[:], sb_raw[:])
        sb_dummy = spool.tile([P, 1, 2], mybir.dt.float32)
        nc.scalar.copy(sb_dummy[:], sb[:, 0:1, :])

        # Prefetch throttle: tile tt+1's x loads wait until tile tt is half
        # consumed by PE, so a streaming tile never fair-shares HBM with the
        # tile PE is working from (contention there stalls PE and makes the
        # HAM clock gate oscillate).
        prefetch_gate = [None] * T_TILES

        for tt in range(T_TILES):
            xt = xpool.tile([P, KI, T_TILE], mybir.dt.bfloat16)
            if tt == 0:
                # Startup: ki-granular interleaved weight/x chunk pairs so the
                # first matching pair lands ~3 us after dispatch instead of
                # the whole 8 MB fair-sharing HBM and landing together.
                # qt on ACT's DGE, xt on SP's DGE to parallelize dispatch.
                for kc in range(KI):
                    nc.scalar.dma_start(qt[:, kc:kc + 1, :], qT_t[:, kc:kc + 1, :])
                    nc.sync.dma_start(
                        xt[:, kc:kc + 1, :], xT_t[:, kc:kc + 1, ts(tt, T_TILE)]
                    )
            else:
                for kc in range(0, KI, 2 * DMA_KC):
                    dma = nc.sync.dma_start(
                        xt[:, kc:kc + 2 * DMA_KC, :],
                        xT_t[:, kc:kc + 2 * DMA_KC, ts(tt, T_TILE)],
                    )
                    if prefetch_gate[tt - 1] is not None:
                        tile.add_dep_helper(
                            dma.ins, prefetch_gate[tt - 1].ins,
                            reason="throttle x prefetch behind prev tile",
                        )
            # ki-outer / ot-inner: the four PSUM groups of this t-tile fill
            # concurrently, so each arriving ki-chunk of x feeds 4 matmuls
            # and PE never starves while the tile streams in. The last tile
            # goes ot-outer instead so three of its four epilogues (and
            # stores) retire before the final matmul, shrinking the tail.
            pss = [
                pspool.tile([P, T_TILE], mybir.dt.float32, tag="ps",
                            name=f"ps_{tt}_{ot}")
                for ot in range(O_TILES)
            ]
            if tt < T_TILES - 1:
                loop = [(ki, ot) for ki in range(KI) for ot in range(O_TILES)]
            else:
                loop = [(ki, ot) for ot in range(O_TILES) for ki in range(KI)]
            for ki, ot in loop:
                mm = nc.tensor.matmul(
                    pss[ot][:],
                    qt[:, ki:ki + 1, ts(ot, P)],
                    xt[:, ki:ki + 1, :],
                    start=(ki == 0),
                    stop=(ki == KI - 1),
                )
                if ki == KI // 2 and ot == 0:
                    prefetch_gate[tt] = mm
            for ot in range(O_TILES):
                ob = opool.tile([P, T_TILE], mybir.dt.bfloat16)
                nc.scalar.activation(
                    ob[:], pss[ot][:],
                    mybir.ActivationFunctionType.Identity,
                    bias=sb[:, ot:ot + 1, 1:2],
                    scale=sb[:, ot:ot + 1, 0:1],
                )
                nc.sync.dma_start(outT.ap()[ts(ot, P), ts(tt, T_TILE)], ob[:])

    nc.compile()
    return nc


_NC_CACHE = None


def kernel(x, weight_fp8, weight_scale, bias):
    global _NC_CACHE, LAST_RESULT

    x = np.asarray(x, dtype=np.float32).reshape(T, IN)
    weight_fp8 = np.asarray(weight_fp8, dtype=np.float32)
    weight_scale = np.asarray(weight_scale, dtype=np.float32).reshape(OUT, 1)
    bias_np = np.asarray(bias, dtype=np.float32).reshape(OUT, 1)

    # Host marshaling: bf16 casts + transposes for contiguous device DMA.
    xT = np.ascontiguousarray(x.astype(BF16).T)            # [IN, T] bf16
    qT = np.ascontiguousarray(weight_fp8.astype(BF16).T)   # [IN, OUT] bf16 (exact)

    scbias = np.ascontiguousarray(
        np.concatenate([weight_scale, bias_np], axis=1)
    )  # [OUT, 2] f32

    in_maps = []
    for c in range(NCORES):
        osl = slice(c * O_C, (c + 1) * O_C)
        in_maps.append({
            "xT": xT,
            "qT": np.ascontiguousarray(qT[:, osl]),
            "scbias": np.ascontiguousarray(scbias[osl]),
        })

    if _NC_CACHE is None:
        _NC_CACHE = _build_bass()
    nc = _NC_CACHE

    res = run_bass_kernel_spmd(
        nc, in_maps, core_ids=list(range(NCORES)), trace=TRACE,
    )
    LAST_RESULT = res

    outT = np.concatenate([r["outT"] for r in res.results], axis=0)  # [OUT, T]
    out = np.ascontiguousarray(outT.T).reshape(B, S, OUT)            # [B, S, OUT] bf16
    return out


# revision 23
# speedup vs baseline: 1.0407x; 1.0407x over previous
"""FP8Linear forward on 8 Trainium2 NeuronCores (Bass/Tile).

Computes out[b,s,o] = sum_i bf16(x)[b,s,i] * q[o,i] * scale[o] + bias[o]
where q = weight_fp8 (fp8-representable values stored as f32).

Sharding: tensor-parallel over out_features (column parallel). Each of the
8 cores owns a 512-row slice of the weight and computes the full token range
for its output slice; x is replicated.

Device kernel layout (per core):
  - lhsT (stationary) = qT tile [i=128, o=128]   (weight slice, pre-transposed)
  - rhs  (moving)     = xT tile [i=128, t=512]   (x, pre-transposed on host)
  - PSUM accumulates out^T tile [o=128, t=512] f32 over 32 i-chunks
  - ScalarE epilogue: out = Identity(psum * scale[o] + bias[o]) -> bf16
    (dequant scale folded into the epilogue; scale is per-out-channel so it
     lands on PSUM partitions, broadcast along the free/t dim)

Host does layout marshaling only: f32->bf16 casts (bf16(x) is the reference's
own first step; weight_fp8 values are exactly representable in bf16) and
transposes so every device DMA is contiguous-per-partition.
"""

import numpy as np
import ml_dtypes
from contextlib import ExitStack

import concourse.bass as bass
from concourse import bacc
import concourse.mybir as mybir
import concourse.tile as tile
from concourse.bass import ts
from concourse.bass_utils import run_bass_kernel_spmd

BF16 = ml_dtypes.bfloat16

P = 128
B, S, IN, OUT = 2, 2048, 4096, 4096
T = B * S                 # 4096 tokens
NCORES = 8
O_C = OUT // NCORES       # 512 out-features per core
KI = IN // P              # 32 contraction chunks
T_TILE = 512
T_TILES = T // T_TILE     # 8
O_TILES = O_C // P        # 4
DMA_KC = 4                # i-chunks per DMA (512 KB x-chunks)

# Set by test.py to capture a profile; harness path leaves these alone.
TRACE = False
LAST_RESULT = None


def _build_bass():
    nc = bacc.Bacc("TRN2", target_bir_lowering=False, debug=False,
                   num_devices=NCORES)

    xT = nc.dram_tensor("xT", [IN, T], mybir.dt.bfloat16, kind="ExternalInput")
    qT = nc.dram_tensor("qT", [IN, O_C], mybir.dt.bfloat16, kind="ExternalInput")
    # scale and bias packed as [O_C, 2] so one DMA covers both (fewer sem
    # waits on the first Activation, which has a small HW wait budget).
    scbias = nc.dram_tensor("scbias", [O_C, 2], mybir.dt.float32, kind="ExternalInput")
    outT = nc.dram_tensor("outT", [O_C, T], mybir.dt.bfloat16, kind="ExternalOutput")

    # [i, f] -> [ki=128 partitions, ko, f]; per-partition rows are contiguous
    xT_t = xT.ap().rearrange("(ko ki) t -> ki ko t", ki=P)
    qT_t = qT.ap().rearrange("(ko ki) o -> ki ko o", ki=P)
    # [o, 2] -> [p=128 partitions, (o_tile, 2)]; per-o-tile scalars are [128, 1]
    sb_t = scbias.ap().rearrange("(ot p) two -> p ot two", p=P)

    with ExitStack() as ctx:
        tc = ctx.enter_context(tile.TileContext(nc))
        wpool = ctx.enter_context(tc.tile_pool(name="w", bufs=1))
        spool = ctx.enter_context(tc.tile_pool(name="s", bufs=1))
        xpool = ctx.enter_context(tc.tile_pool(name="x", bufs=2))
        # One slot per output tile: slot reuse would add a WAR wait on the
        # epilogue Activation, which can only encode a single sem wait.
        opool = ctx.enter_context(tc.tile_pool(name="o", bufs=T_TILES * O_TILES))
        pspool = ctx.enter_context(tc.tile_pool(name="ps", bufs=8, space="PSUM"))

        # Weight slice resident in SBUF for the whole kernel (4 MB bf16).
        # Interleave weight chunks with the first x tile's chunks (emitted in
        # the tt==0 iteration below) so PE can start as soon as matching
        # ki-chunks of both are resident.
        qt = wpool.tile([P, KI, O_C], mybir.dt.bfloat16)

        # The AC (Activation) instruction encodes only ONE sem wait, but the
        # epilogue activation depends on both the PSUM matmul group and the
        # scbias load. Stage scbias through two early ScalarE copies: copy1
        # carries the DMA wait; copy2 (a dummy read of sb) carries the
        # same-engine pipeline-hazard wait on copy1. After that, ScalarE's
        # observed clock covers sb, so every epilogue only waits on PE.
        sb_raw = spool.tile([P, O_TILES, 2], mybir.dt.float32)
        nc.gpsimd.dma_start(sb_raw[:], sb_t)
        sb = spool.tile([P, O_TILES, 2], mybir.dt.float32)
        nc.scalar.copy(sb[:], sb_raw[:])
        sb_dummy = spool.tile([P, 1, 2], mybir.dt.float32)
        nc.scalar.copy(sb_dummy[:], sb[:, 0:1, :])

        # Prefetch throttle: tile tt+1's x loads wait until tile tt is half
        # consumed by PE, so a streaming tile never fair-shares HBM with the
        # tile PE is working from (contention there stalls PE and makes the
        # HAM clock gate oscillate).
        prefetch_gate = [None] * T_TILES

        for tt in range(T_TILES):
            xt = xpool.tile([P, KI, T_TILE], mybir.dt.bfloat16)
            if tt == 0:
                # Startup: ki-granular interleaved weight/x chunk pairs so the
                # first matching pair lands ~3 us after dispatch instead of
                # the whole 8 MB fair-sharing HBM and landing together.
                # qt on ACT's DGE, xt on SP's DGE to parallelize dispatch.
                for kc in range(KI):
                    nc.scalar.dma_start(qt[:, kc:kc + 1, :], qT_t[:, kc:kc + 1, :])
                    nc.sync.dma_start(
                        xt[:, kc:kc + 1, :], xT_t[:, kc:kc + 1, ts(tt, T_TILE)]
                    )
            else:
                for kc in range(0, KI, 2 * DMA_KC):
                    dma = nc.sync.dma_start(
                        xt[:, kc:kc + 2 * DMA_KC, :],
                        xT_t[:, kc:kc + 2 * DMA_KC, ts(tt, T_TILE)],
                    )
                    if tt == 1 and prefetch_gate[0] is not None:
                        # Only tile 1 contends with the startup burst
                        # (weights + tile 0); later tiles self-regulate via
                        # the x-pool WAR on slot reuse.
                        tile.add_dep_helper(
                            dma.ins, prefetch_gate[0].ins,
                            reason="throttle tile1 x prefetch behind tile0",
                        )
            # ki-outer / ot-inner: the four PSUM groups of this t-tile fill
            # concurrently, so each arriving ki-chunk of x feeds 4 matmuls
            # and PE never starves while the tile streams in. The last tile
            # goes ot-outer instead so three of its four epilogues (and
            # stores) retire before the final matmul, shrinking the tail.
            pss = [
                pspool.tile([P, T_TILE], mybir.dt.float32, tag="ps",
                            name=f"ps_{tt}_{ot}")
                for ot in range(O_TILES)
            ]
            if tt < T_TILES - 1:
                loop = [(ki, ot) for ki in range(KI) for ot in range(O_TILES)]
            else:
                loop = [(ki, ot) for ot in range(O_TILES) for ki in range(KI)]
            for ki, ot in loop:
                mm = nc.tensor.matmul(
                    pss[ot][:],
                    qt[:, ki:ki + 1, ts(ot, P)],
                    xt[:, ki:ki + 1, :],
                    start=(ki == 0),
                    stop=(ki == KI - 1),
                )
                if ki == 8 and ot == 0:
                    prefetch_gate[tt] = mm
            for ot in range(O_TILES):
                ob = opool.tile([P, T_TILE], mybir.dt.bfloat16)
                nc.scalar.activation(
                    ob[:], pss[ot][:],
                    mybir.ActivationFunctionType.Identity,
                    bias=sb[:, ot:ot + 1, 1:2],
                    scale=sb[:, ot:ot + 1, 0:1],
                )
                nc.sync.dma_start(outT.ap()[ts(ot, P), ts(tt, T_TILE)], ob[:])

    nc.compile()
    return nc


_NC_CACHE = None


def kernel(x, weight_fp8, weight_scale, bias):
    global _NC_CACHE, LAST_RESULT

    x = np.asarray(x, dtype=np.float32).reshape(T, IN)
    weight_fp8 = np.asarray(weight_fp8, dtype=np.float32)
    weight_scale = np.asarray(weight_scale, dtype=np.float32).reshape(OUT, 1)
    bias_np = np.asarray(bias, dtype=np.float32).reshape(OUT, 1)

    # Host marshaling: bf16 casts + transposes for contiguous device DMA.
    xT = np.ascontiguousarray(x.astype(BF16).T)            # [IN, T] bf16
    qT = np.ascontiguousarray(weight_fp8.astype(BF16).T)   # [IN, OUT] bf16 (exact)

    scbias = np.ascontiguousarray(
        np.concatenate([weight_scale, bias_np], axis=1)
    )  # [OUT, 2] f32

    in_maps = []
    for c in range(NCORES):
        osl = slice(c * O_C, (c + 1) * O_C)
        in_maps.append({
            "xT": xT,
            "qT": np.ascontiguousarray(qT[:, osl]),
            "scbias": np.ascontiguousarray(scbias[osl]),
        })

    if _NC_CACHE is None:
        _NC_CACHE = _build_bass()
    nc = _NC_CACHE

    res = run_bass_kernel_spmd(
        nc, in_maps, core_ids=list(range(NCORES)), trace=TRACE,
    )
    LAST_RESULT = res

    outT = np.concatenate([r["outT"] for r in res.results], axis=0)  # [OUT, T]
    out = np.ascontiguousarray(outT.T).reshape(B, S, OUT)            # [B, S, OUT] bf16
    return out


# revision 30
# speedup vs baseline: 1.0623x; 1.0208x over previous
"""FP8Linear forward on 8 Trainium2 NeuronCores (Bass/Tile).

Computes out[b,s,o] = sum_i bf16(x)[b,s,i] * q[o,i] * scale[o] + bias[o]
where q = weight_fp8 (fp8-representable values stored as f32).

Sharding: tensor-parallel over out_features (column parallel). Each of the
8 cores owns a 512-row slice of the weight and computes the full token range
for its output slice; x is replicated.

Device kernel layout (per core):
  - lhsT (stationary) = qT tile [i=128, o=128]   (weight slice, pre-transposed)
  - rhs  (moving)     = xT tile [i=128, t=512]   (x, pre-transposed on host)
  - PSUM accumulates out^T tile [o=128, t=512] f32 over 32 i-chunks
  - ScalarE epilogue: out = Identity(psum * scale[o] + bias[o]) -> bf16
    (dequant scale folded into the epilogue; scale is per-out-channel so it
     lands on PSUM partitions, broadcast along the free/t dim)

Host does layout marshaling only: f32->bf16 casts (bf16(x) is the reference's
own first step; weight_fp8 values are exactly representable in bf16) and
transposes so every device DMA is contiguous-per-partition.
"""

import numpy as np
import ml_dtypes
from contextlib import ExitStack

import concourse.bass as bass
from concourse import bacc
import concourse.mybir as mybir
import concourse.tile as tile
from concourse.bass import ts
from concourse.bass_utils import run_bass_kernel_spmd

BF16 = ml_dtypes.bfloat16

P = 128
B, S, IN, OUT = 2, 2048, 4096, 4096
T = B * S                 # 4096 tokens
NCORES = 8
O_C = OUT // NCORES       # 512 out-features per core
KI = IN // P              # 32 contraction chunks
T_TILE = 512
T_TILES = T // T_TILE     # 8
O_TILES = O_C // P        # 4
DMA_KC = 4                # i-chunks per DMA (512 KB x-chunks)

# Set by test.py to capture a profile; harness path leaves these alone.
TRACE = False
LAST_RESULT = None


def _build_bass():
    nc = bacc.Bacc("TRN2", target_bir_lowering=False, debug=False,
                   num_devices=NCORES)

    xT = nc.dram_tensor("xT", [IN, T], mybir.dt.bfloat16, kind="ExternalInput")
    qT = nc.dram_tensor("qT", [IN, O_C], mybir.dt.bfloat16, kind="ExternalInput")
    # scale and bias packed as [O_C, 2] so one DMA covers both (fewer sem
    # waits on the first Activation, which has a small HW wait budget).
    scbias = nc.dram_tensor("scbias", [O_C, 2], mybir.dt.float32, kind="ExternalInput")
    outT = nc.dram_tensor("outT", [O_C, T], mybir.dt.bfloat16, kind="ExternalOutput")

    # [i, f] -> [ki=128 partitions, ko, f]; per-partition rows are contiguous
    xT_t = xT.ap().rearrange("(ko ki) t -> ki ko t", ki=P)
    qT_t = qT.ap().rearrange("(ko ki) o -> ki ko o", ki=P)
    # [o, 2] -> [p=128 partitions, (o_tile, 2)]; per-o-tile scalars are [128, 1]
    sb_t = scbias.ap().rearrange("(ot p) two -> p ot two", p=P)

    with ExitStack() as ctx:
        tc = ctx.enter_context(tile.TileContext(nc))
        wpool = ctx.enter_context(tc.tile_pool(name="w", bufs=1))
        spool = ctx.enter_context(tc.tile_pool(name="s", bufs=1))
        xpool = ctx.enter_context(tc.tile_pool(name="x", bufs=2))
        # One slot per output tile: slot reuse would add a WAR wait on the
        # epilogue Activation, which can only encode a single sem wait.
        opool = ctx.enter_context(tc.tile_pool(name="o", bufs=T_TILES * O_TILES))
        pspool = ctx.enter_context(tc.tile_pool(name="ps", bufs=8, space="PSUM"))

        # Weight slice resident in SBUF for the whole kernel (4 MB bf16).
        # Interleave weight chunks with the first x tile's chunks (emitted in
        # the tt==0 iteration below) so PE can start as soon as matching
        # ki-chunks of both are resident.
        qt = wpool.tile([P, KI, O_C], mybir.dt.bfloat16)

        # The AC (Activation) instruction encodes only ONE sem wait, but the
        # epilogue activation depends on both the PSUM matmul group and the
        # scbias load. Stage scbias through two early ScalarE copies: copy1
        # carries the DMA wait; copy2 (a dummy read of sb) carries the
        # same-engine pipeline-hazard wait on copy1. After that, ScalarE's
        # observed clock covers sb, so every epilogue only waits on PE.
        sb_raw = spool.tile([P, O_TILES, 2], mybir.dt.float32)
        nc.gpsimd.dma_start(sb_raw[:], sb_t)
        sb = spool.tile([P, O_TILES, 2], mybir.dt.float32)
        nc.scalar.copy(sb[:], sb_raw[:])
        sb_dummy = spool.tile([P, 1, 2], mybir.dt.float32)
        nc.scalar.copy(sb_dummy[:], sb[:, 0:1, :])

        for tt in range(T_TILES):
            xt = xpool.tile([P, KI, T_TILE], mybir.dt.bfloat16)
            if tt == 0:
                # Startup: 256 KB interleaved weight/x chunk pairs balance
                # queue-head latency (first pairs land early) against
                # per-chunk DMA overhead.
                for kc in range(0, KI, 2):
                    nc.sync.dma_start(qt[:, kc:kc + 2, :], qT_t[:, kc:kc + 2, :])
                    nc.sync.dma_start(
                        xt[:, kc:kc + 2, :], xT_t[:, kc:kc + 2, ts(tt, T_TILE)]
                    )
            else:
                for kc in range(0, KI, DMA_KC):
                    nc.sync.dma_start(
                        xt[:, kc:kc + DMA_KC, :],
                        xT_t[:, kc:kc + DMA_KC, ts(tt, T_TILE)],
                    )
            # ki-outer / ot-inner: the four PSUM groups of this t-tile fill
            # concurrently, so each arriving ki-chunk of x feeds 4 matmuls
            # and PE never starves while the tile streams in. The last tile
            # goes ot-outer instead so three of its four epilogues (and
            # stores) retire before the final matmul, shrinking the tail.
            pss = [
                pspool.tile([P, T_TILE], mybir.dt.float32, tag="ps",
                            name=f"ps_{tt}_{ot}")
                for ot in range(O_TILES)
            ]
            if tt < T_TILES - 1:
                loop = [(ki, ot) for ki in range(KI) for ot in range(O_TILES)]
            else:
                # Last tile: ot-outer so three of the four epilogues (and
                # their stores) retire before the final matmul group ends.
                loop = [(ki, ot) for ot in range(O_TILES) for ki in range(KI)]
            for ki, ot in loop:
                nc.tensor.matmul(
                    pss[ot][:],
                    qt[:, ki:ki + 1, ts(ot, P)],
                    xt[:, ki:ki + 1, :],
                    start=(ki == 0),
                    stop=(ki == KI - 1),
                )
            for ot in range(O_TILES):
                ob = opool.tile([P, T_TILE], mybir.dt.bfloat16)
                nc.scalar.activation(
                    ob[:], pss[ot][:],
                    mybir.ActivationFunctionType.Identity,
                    bias=sb[:, ot:ot + 1, 1:2],
                    scale=sb[:, ot:ot + 1, 0:1],
                )
                nc.sync.dma_start(outT.ap()[ts(ot, P), ts(tt, T_TILE)], ob[:])

    nc.compile()
    return nc


_NC_CACHE = None


def kernel(x, weight_fp8, weight_scale, bias):
    global _NC_CACHE, LAST_RESULT

    x = np.asarray(x, dtype=np.float32).reshape(T, IN)
    weight_fp8 = np.asarray(weight_fp8, dtype=np.float32)
    weight_scale = np.asarray(weight_scale, dtype=np.float32).reshape(OUT, 1)
    bias_np = np.asarray(bias, dtype=np.float32).reshape(OUT, 1)

    # Host marshaling: bf16 casts + transposes for contiguous device DMA.
    xT = np.ascontiguousarray(x.astype(BF16).T)            # [IN, T] bf16
    qT = np.ascontiguousarray(weight_fp8.astype(BF16).T)   # [IN, OUT] bf16 (exact)

    scbias = np.ascontiguousarray(
        np.concatenate([weight_scale, bias_np], axis=1)
    )  # [OUT, 2] f32

    in_maps = []
    for c in range(NCORES):
        osl = slice(c * O_C, (c + 1) * O_C)
        in_maps.append({
            "xT": xT,
            "qT": np.ascontiguousarray(qT[:, osl]),
            "scbias": np.ascontiguousarray(scbias[osl]),
        })

    if _NC_CACHE is None:
        _NC_CACHE = _build_bass()
    nc = _NC_CACHE

    res = run_bass_kernel_spmd(
        nc, in_maps, core_ids=list(range(NCORES)), trace=TRACE,
    )
    LAST_RESULT = res

    outT = np.concatenate([r["outT"] for r in res.results], axis=0)  # [OUT, T]
    out = np.ascontiguousarray(outT.T).reshape(B, S, OUT)            # [B, S, OUT] bf16
    return out


# revision 32
# speedup vs baseline: 1.0665x; 1.0040x over previous
"""FP8Linear forward on 8 Trainium2 NeuronCores (Bass/Tile).

Computes out[b,s,o] = sum_i bf16(x)[b,s,i] * q[o,i] * scale[o] + bias[o]
where q = weight_fp8 (fp8-representable values stored as f32).

Sharding: tensor-parallel over out_features (column parallel). Each of the
8 cores owns a 512-row slice of the weight and computes the full token range
for its output slice; x is replicated.

Device kernel layout (per core):
  - lhsT (stationary) = qT tile [i=128, o=128]   (weight slice, pre-transposed)
  - rhs  (moving)     = xT tile [i=128, t=512]   (x, pre-transposed on host)
  - PSUM accumulates out^T tile [o=128, t=512] f32 over 32 i-chunks
  - ScalarE epilogue: out = Identity(psum * scale[o] + bias[o]) -> bf16
    (dequant scale folded into the epilogue; scale is per-out-channel so it
     lands on PSUM partitions, broadcast along the free/t dim)

Host does layout marshaling only: f32->bf16 casts (bf16(x) is the reference's
own first step; weight_fp8 values are exactly representable in bf16) and
transposes so every device DMA is contiguous-per-partition.
"""

import numpy as np
import ml_dtypes
from contextlib import ExitStack

import concourse.bass as bass
from concourse import bacc
import concourse.mybir as mybir
import concourse.tile as tile
from concourse.bass import ts
from concourse.bass_utils import run_bass_kernel_spmd

BF16 = ml_dtypes.bfloat16

P = 128
B, S, IN, OUT = 2, 2048, 4096, 4096
T = B * S                 # 4096 tokens
NCORES = 8
O_C = OUT // NCORES       # 512 out-features per core
KI = IN // P              # 32 contraction chunks
T_TILE = 512
T_TILES = T // T_TILE     # 8
O_TILES = O_C // P        # 4
DMA_KC = 4                # i-chunks per DMA (512 KB x-chunks)

# Set by test.py to capture a profile; harness path leaves these alone.
TRACE = False
LAST_RESULT = None


def _build_bass():
    nc = bacc.Bacc("TRN2", target_bir_lowering=False, debug=False,
                   num_devices=NCORES)

    xT = nc.dram_tensor("xT", [IN, T], mybir.dt.bfloat16, kind="ExternalInput")
    qT = nc.dram_tensor("qT", [IN, O_C], mybir.dt.bfloat16, kind="ExternalInput")
    # scale and bias packed as [O_C, 2] so one DMA covers both (fewer sem
    # waits on the first Activation, which has a small HW wait budget).
    scbias = nc.dram_tensor("scbias", [O_C, 2], mybir.dt.float32, kind="ExternalInput")
    outT = nc.dram_tensor("outT", [O_C, T], mybir.dt.bfloat16, kind="ExternalOutput")

    # [i, f] -> [ki=128 partitions, ko, f]; per-partition rows are contiguous
    xT_t = xT.ap().rearrange("(ko ki) t -> ki ko t", ki=P)
    qT_t = qT.ap().rearrange("(ko ki) o -> ki ko o", ki=P)
    # [o, 2] -> [p=128 partitions, (o_tile, 2)]; per-o-tile scalars are [128, 1]
    sb_t = scbias.ap().rearrange("(ot p) two -> p ot two", p=P)

    with ExitStack() as ctx:
        tc = ctx.enter_context(tile.TileContext(nc))
        wpool = ctx.enter_context(tc.tile_pool(name="w", bufs=1))
        spool = ctx.enter_context(tc.tile_pool(name="s", bufs=1))
        xpool = ctx.enter_context(tc.tile_pool(name="x", bufs=2))
        # One slot per output tile: slot reuse would add a WAR wait on the
        # epilogue Activation, which can only encode a single sem wait.
        opool = ctx.enter_context(tc.tile_pool(name="o", bufs=T_TILES * O_TILES))
        pspool = ctx.enter_context(tc.tile_pool(name="ps", bufs=8, space="PSUM"))

        # Weight slice resident in SBUF for the whole kernel (4 MB bf16).
        # Interleave weight chunks with the first x tile's chunks (emitted in
        # the tt==0 iteration below) so PE can start as soon as matching
        # ki-chunks of both are resident.
        qt = wpool.tile([P, KI, O_C], mybir.dt.bfloat16)

        # The AC (Activation) instruction encodes only ONE sem wait, but the
        # epilogue activation depends on both the PSUM matmul group and the
        # scbias load. Stage scbias through two early ScalarE copies: copy1
        # carries the DMA wait; copy2 (a dummy read of sb) carries the
        # same-engine pipeline-hazard wait on copy1. After that, ScalarE's
        # observed clock covers sb, so every epilogue only waits on PE.
        sb_raw = spool.tile([P, O_TILES, 2], mybir.dt.float32)
        nc.gpsimd.dma_start(sb_raw[:], sb_t)
        sb = spool.tile([P, O_TILES, 2], mybir.dt.float32)
        nc.scalar.copy(sb[:], sb_raw[:])
        sb_dummy = spool.tile([P, 1, 2], mybir.dt.float32)
        nc.scalar.copy(sb_dummy[:], sb[:, 0:1, :])

        for tt in range(T_TILES):
            xt = xpool.tile([P, KI, T_TILE], mybir.dt.bfloat16)
            if tt == 0:
                # Startup: 256 KB interleaved weight/x chunk pairs balance
                # queue-head latency (first pairs land early) against
                # per-chunk DMA overhead.
                for kc in range(0, KI, 2):
                    nc.sync.dma_start(qt[:, kc:kc + 2, :], qT_t[:, kc:kc + 2, :])
                    nc.sync.dma_start(
                        xt[:, kc:kc + 2, :], xT_t[:, kc:kc + 2, ts(tt, T_TILE)]
                    )
            else:
                for kc in range(0, KI, DMA_KC):
                    nc.sync.dma_start(
                        xt[:, kc:kc + DMA_KC, :],
                        xT_t[:, kc:kc + DMA_KC, ts(tt, T_TILE)],
                    )
            # ki-outer / ot-inner: the four PSUM groups of this t-tile fill
            # concurrently, so each arriving ki-chunk of x feeds 4 matmuls
            # and PE never starves while the tile streams in. The last tile
            # goes ot-outer instead so three of its four epilogues (and
            # stores) retire before the final matmul, shrinking the tail.
            pss = [
                pspool.tile([P, T_TILE], mybir.dt.float32, tag="ps",
                            name=f"ps_{tt}_{ot}")
                for ot in range(O_TILES)
            ]
            if tt < T_TILES - 1:
                loop = [(ki, ot) for ki in range(KI) for ot in range(O_TILES)]
            else:
                # Last tile: ot-outer so three of the four epilogues (and
                # their stores) retire before the final matmul group ends.
                loop = [(ki, ot) for ot in range(O_TILES) for ki in range(KI)]
            for ki, ot in loop:
                nc.tensor.matmul(
                    pss[ot][:],
                    qt[:, ki:ki + 1, ts(ot, P)],
                    xt[:, ki:ki + 1, :],
                    start=(ki == 0),
                    stop=(ki == KI - 1),
                )
            for ot in range(O_TILES):
                ob = opool.tile([P, T_TILE], mybir.dt.bfloat16)
                nc.scalar.activation(
                    ob[:], pss[ot][:],
                    mybir.ActivationFunctionType.Identity,
                    bias=sb[:, ot:ot + 1, 1:2],
                    scale=sb[:, ot:ot + 1, 0:1],
                )
                nc.sync.dma_start(outT.ap()[ts(ot, P), ts(tt, T_TILE)], ob[:])

    nc.compile()
    return nc


_NC_CACHE = None


def kernel(x, weight_fp8, weight_scale, bias):
    global _NC_CACHE, LAST_RESULT

    x = np.asarray(x, dtype=np.float32).reshape(T, IN)
    weight_fp8 = np.asarray(weight_fp8, dtype=np.float32)
    weight_scale = np.asarray(weight_scale, dtype=np.float32).reshape(OUT, 1)
    bias_np = np.asarray(bias, dtype=np.float32).reshape(OUT, 1)

    # Host marshaling: bf16 casts + transposes for contiguous device DMA.
    xT = np.ascontiguousarray(x.astype(BF16).T)            # [IN, T] bf16
    qT = np.ascontiguousarray(weight_fp8.astype(BF16).T)   # [IN, OUT] bf16 (exact)

    scbias = np.ascontiguousarray(
        np.concatenate([weight_scale, bias_np], axis=1)
    )  # [OUT, 2] f32

    in_maps = []
    for c in range(NCORES):
        osl = slice(c * O_C, (c + 1) * O_C)
        in_maps.append({
            "xT": xT,
            "qT": np.ascontiguousarray(qT[:, osl]),
            "scbias": np.ascontiguousarray(scbias[osl]),
        })

    if _NC_CACHE is None:
        _NC_CACHE = _build_bass()
    nc = _NC_CACHE

    res = run_bass_kernel_spmd(
        nc, in_maps, core_ids=list(range(NCORES)), trace=TRACE,
    )
    LAST_RESULT = res

    outT = np.concatenate([r["outT"] for r in res.results], axis=0)  # [OUT, T]
    out = np.ascontiguousarray(outT.T).reshape(B, S, OUT)            # [B, S, OUT] bf16
    return out


# revision 38
# speedup vs baseline: 1.0680x; 1.0014x over previous
"""FP8Linear forward on 8 Trainium2 NeuronCores (Bass/Tile).

Computes out[b,s,o] = sum_i bf16(x)[b,s,i] * q[o,i] * scale[o] + bias[o]
where q = weight_fp8 (fp8-representable values stored as f32).

Sharding: tensor-parallel over out_features (column parallel). Each of the
8 cores owns a 512-row slice of the weight and computes the full token range
for its output slice; x is replicated.

Device kernel layout (per core):
  - lhsT (stationary) = qT tile [i=128, o=128]   (weight slice, pre-transposed)
  - rhs  (moving)     = xT tile [i=128, t=512]   (x, pre-transposed on host)
  - PSUM accumulates out^T tile [o=128, t=512] f32 over 32 i-chunks
  - ScalarE epilogue: out = Identity(psum * scale[o] + bias[o]) -> bf16
    (dequant scale folded into the epilogue; scale is per-out-channel so it
     lands on PSUM partitions, broadcast along the free/t dim)

Host does layout marshaling only: f32->bf16 casts (bf16(x) is the reference's
own first step; weight_fp8 values are exactly representable in bf16) and
transposes so every device DMA is contiguous-per-partition.
"""

import numpy as np
import ml_dtypes
from contextlib import ExitStack

import concourse.bass as bass
from concourse import bacc
import concourse.mybir as mybir
import concourse.tile as tile
from concourse.bass import ts
from concourse.bass_utils import run_bass_kernel_spmd

BF16 = ml_dtypes.bfloat16

P = 128
B, S, IN, OUT = 2, 2048, 4096, 4096
T = B * S                 # 4096 tokens
NCORES = 8
O_C = OUT // NCORES       # 512 out-features per core
KI = IN // P              # 32 contraction chunks
T_TILE = 512
T_TILES = T // T_TILE     # 8
O_TILES = O_C // P        # 4
DMA_KC = 4                # i-chunks per DMA (512 KB x-chunks)

# Set by test.py to capture a profile; harness path leaves these alone.
TRACE = False
LAST_RESULT = None


def _build_bass():
    nc = bacc.Bacc("TRN2", target_bir_lowering=False, debug=False,
                   num_devices=NCORES)

    xT = nc.dram_tensor("xT", [IN, T], mybir.dt.bfloat16, kind="ExternalInput")
    qT = nc.dram_tensor("qT", [IN, O_C], mybir.dt.bfloat16, kind="ExternalInput")
    # scale and bias packed as [O_C, 2] so one DMA covers both (fewer sem
    # waits on the first Activation, which has a small HW wait budget).
    scbias = nc.dram_tensor("scbias", [O_C, 2], mybir.dt.float32, kind="ExternalInput")
    outT = nc.dram_tensor("outT", [O_C, T], mybir.dt.bfloat16, kind="ExternalOutput")

    # [i, f] -> [ki=128 partitions, ko, f]; per-partition rows are contiguous
    xT_t = xT.ap().rearrange("(ko ki) t -> ki ko t", ki=P)
    qT_t = qT.ap().rearrange("(ko ki) o -> ki ko o", ki=P)
    # [o, 2] -> [p=128 partitions, (o_tile, 2)]; per-o-tile scalars are [128, 1]
    sb_t = scbias.ap().rearrange("(ot p) two -> p ot two", p=P)

    with ExitStack() as ctx:
        tc = ctx.enter_context(tile.TileContext(nc))
        wpool = ctx.enter_context(tc.tile_pool(name="w", bufs=1))
        spool = ctx.enter_context(tc.tile_pool(name="s", bufs=1))
        xpool = ctx.enter_context(tc.tile_pool(name="x", bufs=2))
        # One slot per output tile: slot reuse would add a WAR wait on the
        # epilogue Activation, which can only encode a single sem wait.
        opool = ctx.enter_context(tc.tile_pool(name="o", bufs=T_TILES * O_TILES))
        pspool = ctx.enter_context(tc.tile_pool(name="ps", bufs=8, space="PSUM"))

        # Weight slice resident in SBUF for the whole kernel (4 MB bf16).
        # Interleave weight chunks with the first x tile's chunks (emitted in
        # the tt==0 iteration below) so PE can start as soon as matching
        # ki-chunks of both are resident.
        qt = wpool.tile([P, KI, O_C], mybir.dt.bfloat16)

        # The AC (Activation) instruction encodes only ONE sem wait, but the
        # epilogue activation depends on both the PSUM matmul group and the
        # scbias load. Stage scbias through two early ScalarE copies: copy1
        # carries the DMA wait; copy2 (a dummy read of sb) carries the
        # same-engine pipeline-hazard wait on copy1. After that, ScalarE's
        # observed clock covers sb, so every epilogue only waits on PE.
        sb_raw = spool.tile([P, O_TILES, 2], mybir.dt.float32)
        nc.gpsimd.dma_start(sb_raw[:], sb_t)
        sb = spool.tile([P, O_TILES, 2], mybir.dt.float32)
        nc.scalar.copy(sb[:], sb_raw[:])
        sb_dummy = spool.tile([P, 1, 2], mybir.dt.float32)
        nc.scalar.copy(sb_dummy[:], sb[:, 0:1, :])

        # HAM warmup: zero matmuls keep PE busy through its ~3.4us clock-gate
        # window while the first data chunks stream in, so real matmuls start
        # at the full 2.4 GHz instead of ramping from 1.2.
        warm = spool.tile([P, T_TILE], mybir.dt.bfloat16)
        nc.vector.memset(warm[:], 0.0)
        ps_warm = pspool.tile([P, T_TILE], mybir.dt.float32, tag="ps",
                              name="ps_warm")
        N_WARM = 10
        for wi in range(N_WARM):
            nc.tensor.matmul(ps_warm[:], warm[:, 0:P], warm[:, :],
                             start=(wi == 0), stop=(wi == N_WARM - 1))

        for tt in range(T_TILES):
            xt = xpool.tile([P, KI, T_TILE], mybir.dt.bfloat16)
            if tt == 0:
                # Startup: 256 KB interleaved weight/x chunk pairs balance
                # queue-head latency (first pairs land early) against
                # per-chunk DMA overhead.
                for kc in range(0, KI, 2):
                    nc.sync.dma_start(qt[:, kc:kc + 2, :], qT_t[:, kc:kc + 2, :])
                    nc.sync.dma_start(
                        xt[:, kc:kc + 2, :], xT_t[:, kc:kc + 2, ts(tt, T_TILE)]
                    )
            else:
                for kc in range(0, KI, DMA_KC):
                    nc.sync.dma_start(
                        xt[:, kc:kc + DMA_KC, :],
                        xT_t[:, kc:kc + DMA_KC, ts(tt, T_TILE)],
                    )
            # ki-outer / ot-inner: the four PSUM groups of this t-tile fill
            # concurrently, so each arriving ki-chunk of x feeds 4 matmuls
            # and PE never starves while the tile streams in. The last tile
            # goes ot-outer instead so three of its four epilogues (and
            # stores) retire before the final matmul, shrinking the tail.
            pss = [
                pspool.tile([P, T_TILE], mybir.dt.float32, tag="ps",
                            name=f"ps_{tt}_{ot}")
                for ot in range(O_TILES)
            ]
            if tt < T_TILES - 1:
                loop = [(ki, ot) for ki in range(KI) for ot in range(O_TILES)]
            else:
                # Last tile: ot-outer so three of the four epilogues (and
                # their stores) retire before the final matmul group ends.
                loop = [(ki, ot) for ot in range(O_TILES) for ki in range(KI)]
            for ki, ot in loop:
                nc.tensor.matmul(
                    pss[ot][:],
                    qt[:, ki:ki + 1, ts(ot, P)],
                    xt[:, ki:ki + 1, :],
                    start=(ki == 0),
                    stop=(ki == KI - 1),
                )
            for ot in range(O_TILES):
                ob = opool.tile([P, T_TILE], mybir.dt.bfloat16)
                nc.scalar.activation(
                    ob[:], pss[ot][:],
                    mybir.ActivationFunctionType.Identity,
                    bias=sb[:, ot:ot + 1, 1:2],
                    scale=sb[:, ot:ot + 1, 0:1],
                )
                nc.sync.dma_start(outT.ap()[ts(ot, P), ts(tt, T_TILE)], ob[:])

    nc.compile()
    return nc


_NC_CACHE = None


def kernel(x, weight_fp8, weight_scale, bias):
    global _NC_CACHE, LAST_RESULT

    x = np.asarray(x, dtype=np.float32).reshape(T, IN)
    weight_fp8 = np.asarray(weight_fp8, dtype=np.float32)
    weight_scale = np.asarray(weight_scale, dtype=np.float32).reshape(OUT, 1)
    bias_np = np.asarray(bias, dtype=np.float32).reshape(OUT, 1)

    # Host marshaling: bf16 casts + transposes for contiguous device DMA.
    xT = np.ascontiguousarray(x.astype(BF16).T)            # [IN, T] bf16
    qT = np.ascontiguousarray(weight_fp8.astype(BF16).T)   # [IN, OUT] bf16 (exact)

    scbias = np.ascontiguousarray(
        np.concatenate([weight_scale, bias_np], axis=1)
    )  # [OUT, 2] f32

    in_maps = []
    for c in range(NCORES):
        osl = slice(c * O_C, (c + 1) * O_C)
        in_maps.append({
            "xT": xT,
            "qT": np.ascontiguousarray(qT[:, osl]),
            "scbias": np.ascontiguousarray(scbias[osl]),
        })

    if _NC_CACHE is None:
        _NC_CACHE = _build_bass()
    nc = _NC_CACHE

    res = run_bass_kernel_spmd(
        nc, in_maps, core_ids=list(range(NCORES)), trace=TRACE,
    )
    LAST_RESULT = res

    outT = np.concatenate([r["outT"] for r in res.results], axis=0)  # [OUT, T]
    out = np.ascontiguousarray(outT.T).reshape(B, S, OUT)            # [B, S, OUT] bf16
    return out
